# revision 68
# baseline (speedup 1.0000x reference)
"""Trainium2 8-core kernel for the AGI transformer block.

Sharding: 2-way data parallel over batch x 4-way tensor parallel over heads.
Core c: batch b=c//4, feature band g=c%4 (256 features = 4 main heads of 64 /
1 causal head of 256 / 1 meta head of 256).

Precision split by blend weight: the causal path (0.9) and the 0.85 final
out-proj stay bf16; the main path (0.1) and meta path (0.15) run fp8e4m3
with DoubleRow matmuls (two 128-deep contraction subtiles per instruction,
2 MACs/cycle), halving their PE stream time. fp8 operands are pre-scaled
(weights x16, ctx x8, psum casts x2^-k) to sit in e4m3's normal range; the
net scale is folded into the ACT Exp `scale` or the division multiply.

Per core (band slice G = [256g, 256g+256)):
  - main attention: 4 heads, sigmoid(gate+aw) modulation folded into q;
    rowsums via ones-column in the fp8 V (M=65); AV runs DoubleRow over
    j-tile pairs; softmax scale 1/8 folded into the Exp activation.
  - causal MHA head: hd=256 bf16, q pre-scaled 1/16; 0.9 blend folded into
    out-proj weight; main's ctx enters the same PSUM via a DoubleRow
    placement matmul (one-hot x 1/16, ctxm carries 1.6/rs).
  - blend combine: ReduceScatter(add) -> own band (0.85 term) + AllGather
    -> full ctx (meta). Softmax division uses DVE reciprocal_approx_fast
    (no ACT table switches) + a PE ones-matmul partition broadcast.
  - meta MHA head: hd=256 fp8 DoubleRow; 0.15*meta_out_w.T@out_w.T folded
    into one bf16 matrix.
  - final: outP = mowT.T@metaA + owT.T@band_ctx (partial; host sums 4).

Emission interleaves ACT-bound main attention with PE-bound causal attention
and meta projections so the TensorE stream stays dense.
"""

import os

import ml_dtypes
import numpy as np

DEBUG = os.environ.get("KDBG") == "1"

import concourse.mybir as mybir
import concourse.tile as tile
from concourse import bacc
from concourse.bass_utils import run_bass_kernel_spmd

F32 = mybir.dt.float32
BF16 = mybir.dt.bfloat16
F8 = mybir.dt.float8e4
AF = mybir.ActivationFunctionType
MUL = mybir.AluOpType.mult
DR = mybir.MatmulPerfMode.DoubleRow
DP = mybir.MatmulPerfMode.DoublePixel
BF = ml_dtypes.bfloat16
F8NP = ml_dtypes.float8_e4m3

B, S, D = 2, 2048, 1024
NCORES = 8
G = 4  # tensor-parallel group size
BAND = 256  # features per core
IC, NIC = 512, 4  # i-chunk (query) tiling
NJT = 16  # j tiles of 128
NPR = 8  # j-tile pairs per chunk
NKT = 8  # contraction tiles of 128 over D
CAUSAL_ACTIVE = 0.9
MW = ((0.9 - 0.8) / 0.2) * 0.3  # 0.15


def build_program():
    nc = bacc.Bacc("TRN2", target_bir_lowering=False, debug=False,
                   num_devices=NCORES)

    def din(name, shape, dt=BF16):
        return nc.dram_tensor(name, shape, dt, kind="ExternalInput").ap()

    # all weight/x tensors arrive HOST-PACKED as [128, nk, cols] (partition-
    # major) so each load is ONE DMA with a contiguous per-partition segment
    xT = din("xT", [D, S])
    xf8T = din("xf8T", [128, NKT, S], F8)
    wqT = din("wqT", [128, NKT, BAND], F8)
    wkT = din("wkT", [128, NKT, BAND], F8)
    wvT = din("wvT", [128, NKT, 320], F8)  # 4x(64 head cols + ones + pad 80)
    gwT = din("gwT", [128, NKT, 16], F8)  # 4 gate rows + zero pad
    selT = din("selT", [4, 512])  # 4 one-hot row-selector blocks [4,128]
    awc = nc.dram_tensor("awc", [1, 4], F32, kind="ExternalInput").ap()
    cqT = din("cqT", [128, NKT, BAND])
    ckT = din("ckT", [128, NKT, BAND])
    cvT = din("cvT", [128, NKT, BAND])
    cowT = din("cowT", [128, 2, D])
    pcT = din("pcT", [128, 2, D], F8)  # placement matrix (1/16 at own band)
    mqT = din("mqT", [128, NKT, BAND], F8)
    mkT = din("mkT", [128, NKT, BAND], F8)
    mvT = din("mvT", [128, NKT, BAND], F8)
    mowT = din("mowT", [128, 2, D])
    owT = din("owT", [128, 2, D])
    idT = din("idT", [128, 128])
    outP = nc.dram_tensor("outP", [D, S], BF16, kind="ExternalOutput").ap()
    dbg = {}
    if DEBUG:
        for nm, shape, dt in [
            ("d_mrow4", [4, S], BF16), ("d_kf8", [128, 2, S], F8),
            ("d_vsb", [128, NJT, 320], F8), ("d_ctxm", [128, 2, S], F8),
            ("d_cA", [128, 2, S], BF16),
            ("d_ctxF0", [128, NKT, IC], F8), ("d_mq", [128, 2, S], F8),
            ("d_mk", [128, 2, S], F8), ("d_mv", [128, NJT, BAND], F8),
            ("d_mA", [128, 2, S], BF16), ("d_bandC0", [128, 2, IC], BF16),
            ("d_qs", [128, 2, 2 * IC], F8),
        ]:
            dbg[nm] = nc.dram_tensor(nm, shape, dt,
                                     kind="ExternalOutput").ap()

    groups = [[0, 1, 2, 3], [4, 5, 6, 7]]

    with tile.TileContext(nc) as tc:
        with (
            tc.tile_pool(name="wts", bufs=1) as wts,
            tc.tile_pool(name="act", bufs=1) as actp,
            tc.tile_pool(name="small", bufs=1) as small,
            tc.tile_pool(name="work", bufs=3) as work,
            tc.tile_pool(name="stat", bufs=2) as statp,
            tc.tile_pool(name="psE", bufs=2, space="PSUM") as psE,
            tc.tile_pool(name="psA", bufs=4, space="PSUM") as psA,
            tc.tile_pool(name="dram", bufs=1, space="DRAM") as dram,
        ):
            def load_w(name, ap, cols, tag, dt=BF16, q=nc.sync):
                # host-packed [128, NKT, cols]: one contiguous-segment DMA
                t = wts.tile([128, NKT, cols], dt, name=name, tag=tag)
                q.dma_start(t[:, :, :], ap[:, :, :])
                return t

            def load_w2(name, ap, tag, dt=BF16, q=nc.sync):
                # host-packed [128, 2, 1024]: one DMA
                t = wts.tile([128, 2, D], dt, name=name, tag=tag)
                q.dma_start(t[:, :, :], ap[:, :, :])
                return t

            # warmup barrier: absorbs inter-core launch skew while the
            # startup DMAs stream, so later collectives start aligned
            warm_sb = small.tile([1, 4], F32)
            nc.vector.memset(warm_sb[:, :], 0.0)
            warmA = dram.tile([1, 4], F32, name="warmA", tag="warmA")
            warmB = dram.tile([1, 4], F32, name="warmB", tag="warmB")
            nc.sync.dma_start(warmA[:, :], warm_sb[:, :])
            nc.gpsimd.collective_compute(
                "AllReduce", mybir.AluOpType.add, replica_groups=groups,
                ins=[warmA[:, :].opt()], outs=[warmB[:, :].opt()])

            gwf = load_w("gwf", gwT, 16, "gw", F8)
            aw_sb = small.tile([4, 1], F32)
            nc.sync.dma_start(aw_sb[:, :], awc.rearrange("a b -> b a"))
            sel_sb = small.tile([4, 512], BF16)
            nc.sync.dma_start(sel_sb[:, :], selT[:, :])

            # fp8 x in DoubleRow layout, split in two tiles (kt 0-3 / 4-7)
            # whose slots are later reused by the meta ctx fp8 chunks 2/3;
            # one DMA per tile (startup is dispatch-latency bound)
            xf8 = [actp.tile([128, 4, S], F8, name=f"xf8{i}", tag=f"xf8{i}")
                   for i in range(2)]
            # split across two queues so the transfers run in parallel
            nc.sync.dma_start(xf8[0][:, 0:2, :], xf8T[:, 0:2, :])
            nc.sync.dma_start(xf8[0][:, 2:4, :], xf8T[:, 2:4, :])
            nc.gpsimd.dma_start(xf8[1][:, :, :], xf8T[:, 4:8, :])

            def xf8_sl(kp, c0, cw):  # kt-pair kp as [128, 2, cw] slice
                t, r = xf8[kp // 2], (kp % 2) * 2
                return t[:, r:r + 2, c0:c0 + cw]

            wqf = load_w("wqf", wqT, BAND, "wq", F8)
            wkf = load_w("wkf", wkT, BAND, "wk", F8)
            wvf = load_w("wvf", wvT, 320, "wv", F8)

            # bf16 x per-kt tiles (causal path) on the ACT hwdge queue so
            # they don't starve the startup-critical fp8 loads; tags pair
            # them with later-stage tiles so the SBUF slots time-share
            xtags = ["ctxC0", "ctxC1", "ctxC2", "ctxC3",
                     "qT2", "kT2", "vA2", "bandC0"]
            xT_t = []
            for kt in range(NKT):
                t = actp.tile([128, S], BF16, name=f"xTt{kt}", tag=xtags[kt])
                nc.scalar.dma_start(t[:, :], xT[kt * 128:(kt + 1) * 128, :])
                xT_t.append(t)

            # causal weights ride the gpsimd queue (idle until the first
            # collective) to keep the sync queue for startup-critical loads
            cq_sb = load_w("cq_sb", cqT, BAND, "cq", q=nc.gpsimd)
            ck_sb = load_w("ck_sb", ckT, BAND, "ck", q=nc.gpsimd)
            cv_sb = load_w("cv_sb", cvT, BAND, "cv", q=nc.gpsimd)
            ones_sb = small.tile([128, 1], BF16)
            nc.vector.memset(ones_sb[:, :], 1.0)
            onesrow = small.tile([1, 128], BF16)
            nc.vector.memset(onesrow[:, :], 1.0)
            onesrow3 = small.tile([65, 128], BF16)
            nc.vector.memset(onesrow3[:, :], 1.0)
            ones_pair = small.tile([128, 2, 16], F8)
            nc.vector.memset(ones_pair[:, :, :], 1.0)

            def pe_keepwarm(n=8):
                # dependency-free LDWEIGHTS burst across division waits
                for _ in range(n):
                    nc.tensor.ldweights(kf8_sb[:, 0, 0:128])

            # ---------- projections ----------
            qT_sb = actp.tile([128, 2, S], BF16, tag="qT")
            kf8_sb = actp.tile([128, 2, S], F8, tag="kT")

            def proj_chunk_dr(w_f8, ot, icc):
                ps = psA.tile([128, IC], F32, tag="acc")
                for kp in range(4):
                    nc.tensor.matmul(
                        ps[:, :],
                        w_f8[:, 2 * kp:2 * kp + 2, ot * 128:(ot + 1) * 128],
                        xf8_sl(kp, icc * IC, IC),
                        start=(kp == 0), stop=(kp == 3), perf_mode=DR)
                return ps

            def proj_chunk(dst, w_sb, src_t, ot, icc):  # bf16 (causal)
                ps = psA.tile([128, IC], F32, tag="acc")
                for kt in range(NKT):
                    nc.tensor.matmul(
                        ps[:, :],
                        w_sb[:, kt, ot * 128:(ot + 1) * 128],
                        src_t[kt][:, icc * IC:(icc + 1) * IC],
                        start=(kt == 0), stop=(kt == NKT - 1))
                nc.scalar.copy(dst[:, ot, icc * IC:(icc + 1) * IC],
                               ps[:, :])

            def proj_T(dst, w_sb, src_t):
                for ot in range(2):
                    for icc in range(4):
                        proj_chunk(dst, w_sb, src_t, ot, icc)

            # gate matmuls + sigmoid for ALL chunks now (keeps the sigmoid
            # table switch out of the attention blocks)
            mrow4 = small.tile([4, S], BF16)
            for icc in range(4):
                i0 = icc * IC
                g_ps2 = psE.tile([128, 2, IC], F32, tag="eps2")
                g_ps = g_ps2[0:16, 0, :]
                for kp in range(4):
                    nc.tensor.matmul(g_ps[:, :],
                                     gwf[:, 2 * kp:2 * kp + 2, 0:16],
                                     xf8_sl(kp, i0, IC),
                                     start=(kp == 0), stop=(kp == 3),
                                     perf_mode=DR)
                nc.scalar.activation(mrow4[:, i0:i0 + IC], g_ps[0:4, :],
                                     AF.Sigmoid, bias=aw_sb[:, 0:1],
                                     scale=1.0 / 16)

            qs_sb = actp.tile([128, 2, 2 * IC], F8, tag="qs")  # 2-chunk ring

            def qmod(h, icc):
                # broadcast row h of mrow4 to 128 partitions via a K=4 matmul
                # against a one-hot selector, then fold sigma into fp8 q
                rh, oh = (h % 2) * 64, h // 2
                i0 = icc * IC
                pb2 = psE.tile([128, 2, IC], F32, tag="eps2")
                nc.tensor.matmul(pb2[:, 0, :],
                                 sel_sb[0:4, h * 128:(h + 1) * 128],
                                 mrow4[0:4, i0:i0 + IC])
                r0 = (icc % 2) * IC
                nc.vector.tensor_mul(qs_sb[rh:rh + 64, oh, r0:r0 + IC],
                                     qT_sb[rh:rh + 64, oh, i0:i0 + IC],
                                     pb2[rh:rh + 64, 0, :])

            def qproj_steps(icc):
                """q projection/modulation filler steps for chunk icc"""
                steps = []
                for ot in range(2):
                    def sq(ot=ot, icc=icc):
                        ps = proj_chunk_dr(wqf, ot, icc)
                        nc.vector.tensor_scalar_mul(
                            qT_sb[:, ot, icc * IC:(icc + 1) * IC], ps[:, :],
                            1.0 / 16)
                        qmod(2 * ot, icc)
                        qmod(2 * ot + 1, icc)
                    steps.append(sq)
                return steps

            # stage B: only the first-processed chunk of q; full k/v and the
            # whole causal path (keeps xT readers out of the chunk pipeline
            # so the deferred ctxC read-backs never park the sync queue)
            cqT_sb = actp.tile([128, 2, S], BF16, tag="cqT")
            for st_ in qproj_steps(3):
                st_()
            proj_T(cqT_sb, cq_sb, xT_t)
            for ot in range(2):
                for icc in range(4):
                    ps = proj_chunk_dr(wkf, ot, icc)
                    nc.scalar.mul(kf8_sb[:, ot, icc * IC:(icc + 1) * IC],
                                  ps[:, :], 1.0 / 16)

            # v natural layout [2048 j, 320]: head h at cols 80h..80h+63,
            # ones at 80h+64 (written post-copy)
            v_sb = actp.tile([128, NJT, 320], F8, tag="vA")
            for st in range(NJT):
                ps = psA.tile([128, 320], F32, tag="acc")
                for kp in range(4):
                    nc.tensor.matmul(ps[:, :],
                                     xf8_sl(kp, st * 128, 128),
                                     wvf[:, 2 * kp:2 * kp + 2, :],
                                     start=(kp == 0), stop=(kp == 3),
                                     perf_mode=DR)
                nc.scalar.mul(v_sb[:, st, :], ps[:, :], 1.0 / 16)
                nc.vector.memset(v_sb[:, st, 64:320:80], 1.0)

            ckT_sb = actp.tile([128, 2, S], BF16, tag="ckT")
            proj_T(ckT_sb, ck_sb, xT_t)

            cv_nat = actp.tile([128, NJT, BAND], BF16, tag="cvN")
            for st in range(NJT):
                ps = psA.tile([128, BAND], F32, tag="acc")
                for kt in range(NKT):
                    nc.tensor.matmul(ps[:, :],
                                     xT_t[kt][:, st * 128:(st + 1) * 128],
                                     cv_sb[:, kt, :],
                                     start=(kt == 0), stop=(kt == NKT - 1))
                nc.scalar.copy(cv_nat[:, st, :], ps[:, :])

            # late-stage weights: load now (off the startup critical path;
            # slots of wq/wk/wv just freed); gpsimd queue keeps sync free
            mqf = load_w("mqf", mqT, BAND, "wq", F8, q=nc.gpsimd)
            mkf = load_w("mkf", mkT, BAND, "wk", F8, q=nc.gpsimd)
            mvf = load_w("mvf", mvT, BAND, "wv", F8, q=nc.gpsimd)
            cow_sb = load_w2("cow_sb", cowT, "cow", q=nc.gpsimd)
            pcf = load_w2("pcf", pcT, "pc", F8, q=nc.gpsimd)
            mow_sb = load_w2("mow_sb", mowT, "mow", q=nc.gpsimd)
            id_sb = small.tile([128, 128], BF16)
            nc.gpsimd.dma_start(id_sb[:, :], idT[:, :])
            ow_sb = load_w2("ow_sb", owT, "ow", q=nc.gpsimd)

            # ---------- chunked tiles ----------
            ctxm_sb = actp.tile([128, 2, S], F8, tag="ctxm")  # 1.6*main ctx
            cA_sb = actp.tile([128, 2, S], BF16, tag="cA")
            # fp8 ctx (x8) for the meta projections, straight from the fp8
            # AllGather; chunk 2 (last processed) reuses an xf8 slot (dead
            # after the final q projections)
            ctxF = [actp.tile([128, NKT, IC], F8, name=f"ctxF{i}",
                              tag=("ctxF0", "ctxF1", "xf80", "ctxF3")[i])
                    for i in range(NIC)]
            bandC = [actp.tile([128, 2, IC], BF16, name=f"bandC{i}",
                               tag=f"bandC{i}") for i in range(NIC)]
            mqT_f8 = actp.tile([128, 2, S], F8, tag="qT2")
            mkT_f8 = actp.tile([128, 2, S], F8, tag="kT2")
            mv_nat = actp.tile([128, NJT, BAND], F8, tag="vA2")

            # chunks are processed in ORDER so the last collective (chunk 2)
            # is needed only by the tail of the meta phase
            ORDER = [3, 0, 1, 2]
            LASTC = ORDER[-1]
            arB, rsO, agI, agF = [], [], [], []
            for icc in range(NIC):
                nh = 1
                arB.append([dram.tile([D, IC // nh], BF16,
                                      name=f"arB{icc}_{hh}", tag=f"arB{icc}{hh}")
                            for hh in range(nh)])
                rsO.append([dram.tile([BAND, IC // nh], BF16,
                                      name=f"rsO{icc}_{hh}", tag=f"rsO{icc}{hh}")
                            for hh in range(nh)])
                agI.append([dram.tile([128, 2 * (IC // nh)], F8,
                                      name=f"agI{icc}_{hh}", tag=f"agI{icc}{hh}")
                            for hh in range(nh)])
                agF.append([dram.tile([4 * 128, 2 * (IC // nh)], F8,
                                      name=f"agF{icc}_{hh}", tag=f"agF{icc}{hh}")
                            for hh in range(nh)])
            def agchain(icc, hh, c0, hw_):
                # fp8 cast + AllGather + readbacks; the WHOLE chain rides
                # the gpsimd queue so a late RS parks only its own chain
                bcf = work.tile([128, 2, hw_], F8, tag="bcf", bufs=2,
                                name=f"bcf{icc}{hh}")
                nc.gpsimd.tensor_scalar_mul(
                    bcf[:, :, :], bandC[icc][:, 0:2, c0:c0 + hw_], 8.0)
                nc.gpsimd.dma_start(
                    agI[icc][hh][:, :],
                    bcf[:, :, :].rearrange("p a c -> p (a c)"))
                nc.gpsimd.collective_compute(
                    "AllGather", mybir.AluOpType.bypass,
                    replica_groups=groups,
                    ins=[agI[icc][hh][:, :].opt()],
                    outs=[agF[icc][hh][:, :].opt()])
                for g2 in range(4):
                    nc.gpsimd.dma_start(
                        ctxF[icc][:, 2 * g2:2 * g2 + 2, c0:c0 + hw_],
                        agF[icc][hh][g2 * 128:(g2 + 1) * 128, :]
                        .rearrange("p (a c) -> p a c", a=2))

            def div_prep(specs, i0):
                """softmax divisions, split for PE continuity: the DVE
                fast-reciprocal chain is emitted NOW (it only needs the
                rowsums); the PE ones-matmul broadcast + DVE multiplies are
                returned as filler closures to weave into later PE work.
                spec: ("head", h, acc64_ap, rs_ap)
                   or ("wide", dst_sb, a1, a2, rs_ap, sc)"""
                # batched reciprocal: relocate the rowsums to partitions
                # 0/32/64 of ONE tile (legal PE moving-base positions), then
                # a single reciprocal + cast serve every spec — DVE time
                # scales with free size, not partitions, so this is ~2x
                # lower latency for the chain the PE broadcasts wait on
                rs3 = statp.tile([65, IC], F32, tag="rs0", bufs=1)
                for i_, sp in enumerate(specs):
                    rs_ap = sp[3] if sp[0] == "head" else sp[4]
                    nc.vector.tensor_copy(rs3[32 * i_:32 * i_ + 1, :], rs_ap)
                rcp3 = statp.tile([65, IC], F32, tag="lnr", bufs=2)
                nc.vector.reciprocal_approx_fast(rcp3[:, :], rs3[:, :])
                rcpb3 = statp.tile([65, IC], BF16, tag="rcp", bufs=3)
                nc.vector.tensor_copy(rcpb3[:, :], rcp3[:, :])
                rcps = [rcpb3[32 * i_:32 * i_ + 1, :]
                        for i_ in range(len(specs))]

                def one(sp, rcpb, i0=i0):
                    n = 64 if sp[0] == "head" else 128
                    bp = rcpb.base_partition()
                    pb_ps = psE.tile([128, 2, IC], F32, tag="eps2")
                    nc.tensor.matmul(pb_ps[:, 0, :],
                                     onesrow3[bp:bp + 1, :], rcpb)
                    pb = work.tile([n, IC], BF16,
                                   tag="pbm" if n == 64 else "pb2", bufs=3)
                    nc.vector.tensor_copy(pb[:, :], pb_ps[0:n, 0, :])
                    if sp[0] == "head":
                        h, acc64 = sp[1], sp[2]
                        rh, oh = (h % 2) * 64, h // 2
                        nc.vector.scalar_tensor_tensor(
                            ctxm_sb[rh:rh + 64, oh, i0:i0 + IC],
                            acc64, 1.6, pb[:, :], MUL, MUL)
                    else:
                        dst_sb, a1, a2, sc = sp[1], sp[2], sp[3], sp[5]
                        nc.vector.scalar_tensor_tensor(
                            dst_sb[:, 0, i0:i0 + IC], a1[:, :], sc,
                            pb[:, :], MUL, MUL)
                        nc.vector.scalar_tensor_tensor(
                            dst_sb[:, 1, i0:i0 + IC], a2[:, :], sc,
                            pb[:, :], MUL, MUL)
                return [lambda sp=sp, rcpb=rcpb: one(sp, rcpb)
                        for sp, rcpb in zip(specs, rcps)]

            def main_pair_step(p, t, i0, accs, mid=None):
                """jt pair (2t, 2t+1) for main heads (2p, 2p+1): per head one
                2-bank score psum (both jt) -> ONE merged Exp -> DoubleRow AV
                (M=65: v cols + ones-column rowsum at row 64). DR dsts must
                sit at partition 0, hence one [65,IC] bank per head.
                mid() (if given) emits between the scores and the first AV —
                the hook point for deferred division broadcasts that must
                precede the accumulator-claiming AV in the PE FIFO."""
                oh = p
                r0 = (i0 // IC % 2) * IC
                esbs = [work.tile([128, 2, IC], F8, tag="esb2", bufs=4,
                                  name=f"esb{p}{t}{i0}{hh}")
                        for hh in range(2)]
                for hh in range(2):
                    rh = hh * 64
                    eps2 = psE.tile([128, 2, IC], F32, tag="eps2")
                    for dj in range(2):
                        jt = 2 * t + dj
                        nc.tensor.matmul(
                            eps2[:, dj, :],
                            kf8_sb[rh:rh + 64, oh, jt * 128:(jt + 1) * 128],
                            qs_sb[rh:rh + 64, oh, r0:r0 + IC])
                    nc.scalar.activation(esbs[hh][:, 0:2, :], eps2[:, 0:2, :],
                                         AF.Exp, scale=0.125)
                if mid is not None:
                    mid()
                for hh in range(2):
                    h = 2 * p + hh
                    nc.tensor.matmul(
                        accs[hh][:, :],
                        v_sb[:, 2 * t:2 * t + 2, 80 * h:80 * h + 65],
                        esbs[hh][:, 0:2, :],
                        start=(t == 0), stop=(t == NPR - 1), perf_mode=DR)

            def causal_attn_step(t, i0, a1, a2, crs_acc):
                """jt pair (2t, 2t+1) of the bf16 hd-256 causal attention:
                one 2-bank score psum for both jt -> ONE merged Exp; the
                rowsum partials land back in the (exp-consumed) score psum
                and accumulate into SBUF crs_acc on the DVE"""
                eps2 = psE.tile([128, 2, IC], F32, tag="eps2")
                for dj in range(2):
                    jt = 2 * t + dj
                    for dkt in range(2):
                        nc.tensor.matmul(
                            eps2[:, dj, :],
                            ckT_sb[:, dkt, jt * 128:(jt + 1) * 128],
                            cqT_sb[:, dkt, i0:i0 + IC],
                            start=(dkt == 0), stop=(dkt == 1))
                esb = work.tile([128, 2, IC], BF16, tag="esb", bufs=4)
                nc.scalar.activation(esb[:, 0:2, :], eps2[:, 0:2, :], AF.Exp)
                for dj in range(2):
                    jt = 2 * t + dj
                    st_, sp_ = (jt == 0), (jt == NJT - 1)
                    nc.tensor.matmul(a1[:, :], cv_nat[:, jt, 0:128],
                                     esb[:, dj, :], start=st_, stop=sp_)
                    nc.tensor.matmul(a2[:, :], cv_nat[:, jt, 128:256],
                                     esb[:, dj, :], start=st_, stop=sp_)
                    nc.tensor.matmul(eps2[0:1, 0, :], ones_sb[:, 0:1],
                                     esb[:, dj, :],
                                     start=(dj == 0), stop=(dj == 1))
                nc.vector.tensor_add(crs_acc[0:1, :], crs_acc[0:1, :],
                                     eps2[0:1, 0, :])

            def meta_attn_step(t, i0, a1, a2, rs, st_, sp_, mid=None,
                               st_rs=None):
                if st_rs is None:
                    st_rs = st_
                """jt pair (2t, 2t+1) of the fp8 DoubleRow meta attention;
                one 2-bank score psum for both jt -> ONE merged Exp"""
                esbm = work.tile([128, 2, IC], F8, tag="esbm", bufs=3)
                eps2 = psE.tile([128, 2, IC], F32, tag="eps2")
                for dj in range(2):
                    jt = 2 * t + dj
                    nc.tensor.matmul(
                        eps2[:, dj, :],
                        mkT_f8[:, 0:2, jt * 128:(jt + 1) * 128],
                        mqT_f8[:, 0:2, i0:i0 + IC], perf_mode=DR)
                nc.scalar.activation(esbm[:, 0:2, :], eps2[:, 0:2, :],
                                     AF.Exp, scale=1.0 / 256)
                if mid is not None:
                    mid()
                nc.tensor.matmul(a1[:, :], mv_nat[:, 2 * t:2 * t + 2, 0:128],
                                 esbm[:, 0:2, :], start=st_, stop=sp_,
                                 perf_mode=DR)
                nc.tensor.matmul(a2[:, :], mv_nat[:, 2 * t:2 * t + 2, 128:256],
                                 esbm[:, 0:2, :], start=st_, stop=sp_,
                                 perf_mode=DR)
                nc.tensor.matmul(rs[:, :], ones_pair[:, 0:2, 0:1],
                                 esbm[:, 0:2, :], start=st_rs, stop=sp_,
                                 perf_mode=DR)

            def metaproj_steps(icc):
                """closures emitting chunk icc's meta projections (fp8 DR)"""
                i0 = icc * IC
                steps = []


                def projstep(w_f8, dst, ot, i0=i0, icc=icc):
                    ps = psA.tile([128, IC], F32, tag="acc")
                    for kp in range(4):
                        nc.tensor.matmul(
                            ps[:, :],
                            w_f8[:, 2 * kp:2 * kp + 2,
                                 ot * 128:(ot + 1) * 128],
                            ctxF[icc][:, 2 * kp:2 * kp + 2, :],
                            start=(kp == 0), stop=(kp == 3), perf_mode=DR)
                    nc.vector.tensor_scalar_mul(dst[:, ot, i0:i0 + IC],
                                                ps[:, :], 2.0 ** -5)

                def vstep(st4, icc=icc):
                    st = icc * 4 + st4
                    ps = psA.tile([128, BAND], F32, tag="acc")
                    for kp in range(4):
                        nc.tensor.matmul(
                            ps[:, :],
                            ctxF[icc][:, 2 * kp:2 * kp + 2,
                                      st4 * 128:(st4 + 1) * 128],
                            mvf[:, 2 * kp:2 * kp + 2, 0:BAND],
                            start=(kp == 0), stop=(kp == 3), perf_mode=DR)
                    nc.vector.tensor_scalar_mul(mv_nat[:, st, :], ps[:, :],
                                                2.0 ** -5)

                for ot in range(2):
                    steps.append(lambda ot=ot: projstep(mqf, mqT_f8, ot))
                for ot in range(2):
                    steps.append(lambda ot=ot: projstep(mkf, mkT_f8, ot))
                for st4 in range(4):
                    steps.append(lambda st4=st4: vstep(st4))
                return steps

            # ---------- per-chunk pipeline ----------
            # the fp8 cast + AllGather chain of chunk X is deferred into
            # chunk X+1's phase 2, when X's ReduceScatter is long done —
            # the DVE cast then never parks the strict-FIFO Vector queue
            pending_ag = []
            for oi, icc in enumerate(ORDER):
                i0 = icc * IC
                # phase 1: main heads (0,1) interleaved with causal attention
                accA = [psA.tile([65, IC], F32, tag="acc",
                                 name=f"accA{icc}{i}") for i in range(2)]
                ca1 = psA.tile([128, IC], F32, tag="acc")
                ca2 = psA.tile([128, IC], F32, tag="acc")
                crs_acc = statp.tile([1, IC], F32, tag="crsa", bufs=1,
                                     name=f"crsa{icc}")
                nc.vector.memset(crs_acc[:, :], 0.0)
                for t in range(NPR):
                    main_pair_step(0, t, i0, accA)
                    causal_attn_step(t, i0, ca1, ca2, crs_acc)
                # DVE reciprocal chain starts now; PE broadcasts deferred
                div1 = div_prep(
                    [("head", 0, accA[0][0:64, :], accA[0][64:65, :]),
                     ("head", 1, accA[1][0:64, :], accA[1][64:65, :]),
                     ("wide", cA_sb, ca1, ca2, crs_acc[0:1, :], 1.0)], i0)

                # phase 2: main heads (2,3) interleaved with filler PE work:
                # next chunk's q/cq projections + older chunk's meta projs
                accB = [psA.tile([65, IC], F32, tag="acc",
                                 name=f"accB{icc}{i}") for i in range(2)]
                fillers = []
                if oi + 1 < NIC:
                    fillers += qproj_steps(ORDER[oi + 1])
                while pending_ag:
                    pending_ag.pop(0)()
                nfront = len(fillers)
                done = 0
                for t in range(NPR):
                    # pair 0 emits the phase-1 division broadcasts between
                    # its scores and its accumulator-claiming AVs: the AV's
                    # WAR on the accA slots resolves through the divisions'
                    # DVE multiplies, which must precede it in the PE FIFO
                    mid = (lambda: [st_() for st_ in div1]) if t == 0 else None
                    main_pair_step(1, t, i0, accB, mid=mid)
                    # front-load fillers (drain by pair 3): the next chunk's
                    # phase-1 scores need the qmod outputs, and phase 2's
                    # tail is ACT-bound anyway so late-phase PE idle is free
                    want = min(nfront, (t + 1) * ((nfront + 2) // 3))
                    while done < want:
                        fillers[done]()
                        done += 1
                div2 = div_prep(
                    [("head", 2, accB[0][0:64, :], accB[0][64:65, :]),
                     ("head", 3, accB[1][0:64, :], accB[1][64:65, :])], i0)

                # causal out-proj + DoubleRow main placement -> arB chunk.
                # cow-only groups open first (they need only cA); the pc
                # close (needs ctxm h2/h3) trails by one ot so the phase-2
                # division broadcasts hide behind cow matmuls
                def close_ot(ot, ps):
                    nc.tensor.matmul(
                        ps[:, :],
                        pcf[:, 0:2, ot * 128:(ot + 1) * 128],
                        ctxm_sb[:, 0:2, i0:i0 + IC],
                        start=False, stop=True, perf_mode=DR)
                    ob = work.tile([128, IC], BF16, tag="obA", bufs=2)
                    nc.vector.tensor_copy(ob[:, :], ps[:, :])
                    nh = len(arB[icc])
                    hw_ = IC // nh
                    for hh in range(nh):
                        nc.sync.dma_start(
                            arB[icc][hh][ot * 128:(ot + 1) * 128, :],
                            ob[:, hh * hw_:(hh + 1) * hw_])

                if True:
                    opens = []
                    for ot in range(8):
                        ps = psA.tile([128, IC], F32, tag="acc")
                        for ft in range(2):
                            nc.tensor.matmul(
                                ps[:, :],
                                cow_sb[:, ft, ot * 128:(ot + 1) * 128],
                                cA_sb[:, ft, i0:i0 + IC],
                                start=(ft == 0), stop=False)
                        opens.append((ot, ps))
                        if ot < 2:
                            div2[ot]()
                        else:
                            close_ot(*opens.pop(0))
                        if done < len(fillers) and ot % 2 == 1:
                            fillers[done]()
                            done += 1
                    while opens:
                        close_ot(*opens.pop(0))
                else:
                    # LAST chunk: column-split outproj so half 0's stores
                    # finish early and its RS launches mid-outproj; the two
                    # half-chains (RS -> cast -> AG) then pipeline, roughly
                    # halving the exposed latency at the meta seam
                    def close_half(ot, ps, hf):
                        c0h = i0 + hf * 256
                        nc.tensor.matmul(
                            ps[:, :],
                            pcf[:, 0:2, ot * 128:(ot + 1) * 128],
                            ctxm_sb[:, 0:2, c0h:c0h + 256],
                            start=False, stop=True, perf_mode=DR)
                        ob = work.tile([128, 256], BF16, tag="obA", bufs=2)
                        nc.vector.tensor_copy(ob[:, :], ps[:, :])
                        nc.sync.dma_start(
                            arB[icc][hf][ot * 128:(ot + 1) * 128, :],
                            ob[:, :])
                    for hf in range(2):
                        c0h = i0 + hf * 256
                        opens = []
                        for ot in range(8):
                            ps = psA.tile([128, 256], F32, tag="acc")
                            for ft in range(2):
                                nc.tensor.matmul(
                                    ps[:, :],
                                    cow_sb[:, ft, ot * 128:(ot + 1) * 128],
                                    cA_sb[:, ft, c0h:c0h + 256],
                                    start=(ft == 0), stop=False)
                            opens.append((ot, ps))
                            if hf == 0 and ot < 2:
                                div2[ot]()
                            elif len(opens) > 1:
                                close_half(*opens.pop(0), hf)
                        while opens:
                            close_half(*opens.pop(0), hf)
                        nc.gpsimd.collective_compute(
                            "ReduceScatter", mybir.AluOpType.add,
                            replica_groups=groups,
                            ins=[arB[icc][hf][:, :].opt()],
                            outs=[rsO[icc][hf][:, :].opt()])
                        for kt in range(2):
                            nc.gpsimd.dma_start(
                                bandC[icc][:, kt, hf * 256:hf * 256 + 256],
                                rsO[icc][hf][kt * 128:(kt + 1) * 128, :])
                        pending_ag.append(
                            lambda icc=icc, hf=hf: agchain(icc, hf,
                                                           hf * 256, 256))

                # blend combine: ReduceScatter (bf16 own band -> bandC) then
                # AllGather of the fp8-cast band (half the bytes -> ctxF).
                # The last chunk is split in half for a shorter latency tail.
                # Read-backs ride the gpsimd queue: parking it on a
                # collective's completion is free (the CC engine is serial
                # anyway) and the sync queue stays park-free for the stores.
                if True:
                    nc.gpsimd.collective_compute(
                        "ReduceScatter", mybir.AluOpType.add,
                        replica_groups=groups,
                        ins=[arB[icc][0][:, :].opt()],
                        outs=[rsO[icc][0][:, :].opt()])
                    for kt in range(2):
                        nc.gpsimd.dma_start(
                            bandC[icc][:, kt, :],
                            rsO[icc][0][kt * 128:(kt + 1) * 128, :])
                    pending_ag.append(
                        lambda icc=icc: agchain(icc, 0, 0, IC))

            # ALL meta projections emit after the chunk pipeline: emitting
            # them as phase-2 fillers head-blocks the PE FIFO on the
            # collective-derived ctxF whenever cores drift apart. Chunk 2's
            # are emitted inside the first meta attention chunk below.
            while pending_ag:
                pending_ag.pop(0)()
            for c_ in (ORDER[0], ORDER[1], ORDER[2]):
                for st in metaproj_steps(c_):
                    st()

            # ---------- meta attention + final out-proj ----------
            def final_steps(icc):
                i0 = icc * IC
                steps = []

                def fstep(ot, icc=icc, i0=i0):
                    ps = psA.tile([128, IC], F32, tag="acc")
                    for ft in range(2):
                        nc.tensor.matmul(
                            ps[:, :],
                            mow_sb[:, ft, ot * 128:(ot + 1) * 128],
                            mA_sb[:, ft, i0:i0 + IC],
                            start=(ft == 0), stop=False)
                    for ft in range(2):
                        nc.tensor.matmul(
                            ps[:, :],
                            ow_sb[:, ft, ot * 128:(ot + 1) * 128],
                            bandC[icc][:, ft, :],
                            start=False, stop=(ft == 1))
                    ob = work.tile([128, IC], BF16, tag="obF", bufs=2)
                    nc.vector.tensor_copy(ob[:, :], ps[:, :])
                    # gpsimd dispatch is ~25ns vs sync's 565ns and the
                    # queue is idle once the collectives are done
                    nc.gpsimd.dma_start(
                        outP[ot * 128:(ot + 1) * 128, i0:i0 + IC], ob[:, :])
                for ot in range(8):
                    steps.append(lambda ot=ot: fstep(ot))
                return steps

            # j-pair order matches ctx availability order (chunk 2 last)
            PAIRS = [6, 7, 0, 1, 2, 3, 4, 5]
            mA_sb = actp.tile([128, 2, S], BF16, tag="cqT")  # reuse slot

            # PASS A — chain-free coverage: for q-chunks 3,0,1 run the six
            # j-pairs that avoid j-chunk 2, partial-close, and stage the
            # partial sums to SBUF (bf16; 0.15-weighted meta tolerates it).
            # This work fills the window while chunk 2's RS/AG chain flies.
            stageA = {}
            for c_ in (ORDER[0], ORDER[1], ORDER[2]):
                i0 = c_ * IC
                a1 = psA.tile([128, IC], F32, tag="acc")
                a2 = psA.tile([128, IC], F32, tag="acc")
                rs = psA.tile([1, IC], F32, tag="acc", name="mrs")
                for pi in range(6):
                    meta_attn_step(PAIRS[pi], i0, a1, a2, rs,
                                   pi == 0, pi == 5)
                sg = work.tile([128, 2, IC], BF16, tag="esb", bufs=4,
                               name=f"mstg{c_}")
                sgr = statp.tile([1, IC], F32, tag="mstgr", bufs=3,
                                 name=f"mstgr{c_}")
                nc.vector.tensor_copy(sg[:, 0, :], a1[:, :])
                nc.vector.tensor_copy(sg[:, 1, :], a2[:, :])
                nc.vector.tensor_copy(sgr[:, :], rs[:, :])
                stageA[c_] = (sg, sgr)

            # chunk 2's meta projections — first consumer of the collective
            for st in metaproj_steps(LASTC):
                st()

            # PASS B — per q-chunk: reinject staged partials (identity
            # matmul re-seeds the psum), finish the j-chunk-2 pairs, divide,
            # and interleave the previous chunk's final out-proj steps
            # q-chunk 2 (the only one needing a full 8-pair pass) goes
            # FIRST: it is ready the moment metaproj(2) lands, and the tail
            # then ends on a 2-pair reinjected chunk instead of 8 pairs
            PB = [LASTC] + [c for c in ORDER if c != LASTC]
            pend_div = []
            for mi, icc in enumerate(PB):
                i0 = icc * IC
                fsteps = final_steps(PB[mi - 1]) if mi > 0 else []
                for st_ in pend_div:
                    st_()
                pend_div = []
                a1 = psA.tile([128, IC], F32, tag="acc")
                a2 = psA.tile([128, IC], F32, tag="acc")
                rs = psA.tile([1, IC], F32, tag="acc", name="mrs")
                if icc == LASTC:
                    for pi in range(NPR):
                        meta_attn_step(PAIRS[pi], i0, a1, a2, rs,
                                       pi == 0, pi == NPR - 1)
                        if fsteps and pi < len(fsteps):
                            fsteps[pi]()
                    fsteps = fsteps[NPR:]
                    rs_ap = rs[0:1, :]
                else:
                    sg, sgr = stageA[icc]
                    nc.tensor.matmul(a1[:, :], id_sb[:, :], sg[:, 0, :],
                                     start=True, stop=False)
                    nc.tensor.matmul(a2[:, :], id_sb[:, :], sg[:, 1, :],
                                     start=True, stop=False)
                    if fsteps:
                        fsteps[0]()
                    for pi in range(6, NPR):
                        meta_attn_step(PAIRS[pi], i0, a1, a2, rs,
                                       False, pi == NPR - 1,
                                       st_rs=(pi == 6))
                        if len(fsteps) > pi - 5:
                            fsteps[pi - 5]()
                    fsteps = fsteps[3:]
                    # merged rowsum: staged partial + j-chunk-2 psum part
                    rs_m = statp.tile([1, IC], F32, tag="mrsm", bufs=1,
                                      name=f"mrsm{icc}")
                    nc.vector.tensor_add(rs_m[0:1, :], sgr[0:1, :],
                                         rs[0:1, :])
                    rs_ap = rs_m[0:1, :]
                for fs in fsteps:
                    fs()
                pend_div = div_prep([("wide", mA_sb, a1, a2, rs_ap, 0.25)],
                                    i0)

            for st_ in pend_div:
                st_()
            for st in final_steps(PB[-1]):
                st()

            if DEBUG:
                for nm, t in [
                    ("d_mrow4", mrow4), ("d_kf8", kf8_sb), ("d_vsb", v_sb),
                    ("d_ctxm", ctxm_sb), ("d_cA", cA_sb),
                    ("d_ctxF0", ctxF[0]),
                    ("d_mq", mqT_f8), ("d_mk", mkT_f8), ("d_mv", mv_nat),
                    ("d_mA", mA_sb), ("d_bandC0", bandC[0]),
                    ("d_qs", qs_sb),
                ]:
                    ap = dbg[nm]
                    if len(t.shape) == 2:
                        nc.sync.dma_start(ap[:, :], t[:, :])
                    else:
                        nc.sync.dma_start(ap[:, :, :], t[:, :, :])

    nc.compile()
    return nc


_NC = None


def _get_nc():
    global _NC
    if _NC is None:
        _NC = build_program()
    return _NC


def kernel(hidden_states, consciousness_vector, wq, bq, wk, bk, wv, bv,
           gate_w, gate_b, aw_w, aw_b,
           causal_in_w, causal_in_b, causal_out_w, causal_out_b,
           meta_in_w, meta_in_b, meta_out_w, meta_out_b,
           out_w, out_b):
    f = np.float32
    hs = np.asarray(hidden_states, f)
    aw = np.asarray(consciousness_vector, f) @ np.asarray(aw_w, f).T \
        + np.asarray(aw_b, f)
    wfused = np.asarray(meta_out_w, f).T @ np.asarray(out_w, f).T  # [D, D]
    xTs = [np.ascontiguousarray(hs[b].T) for b in range(B)]

    def bfT(a):  # transpose + bf16
        return np.ascontiguousarray(np.asarray(a, f).T).astype(BF)

    def f8T(a, scale=16.0):  # transpose + scale + fp8
        return np.ascontiguousarray(np.asarray(a, f).T * scale).astype(F8NP)

    def pack8(a):  # [1024, cols] -> [128, 8, cols] partition-major
        return np.ascontiguousarray(
            np.asarray(a).reshape(8, 128, -1).transpose(1, 0, 2))

    def pack2(a):  # [256, 1024] -> [128, 2, 1024] partition-major
        return np.ascontiguousarray(
            np.asarray(a).reshape(2, 128, -1).transpose(1, 0, 2))

    in_maps = []
    for c in range(NCORES):
        b, g = c // G, c % G
        sl = slice(g * BAND, (g + 1) * BAND)
        wv_aug = np.zeros((D, 320), f)
        for h in range(4):
            wv_aug[:, h * 80:h * 80 + 64] = \
                16.0 * np.asarray(wv, f)[g * BAND + h * 64:
                                         g * BAND + (h + 1) * 64].T
        gw_aug = np.zeros((D, 16), f)
        gw_aug[:, 0:4] = 16.0 * np.asarray(gate_w, f)[4 * g:4 * g + 4].T
        sel4 = np.zeros((4, 512), f)
        for h in range(4):
            sel4[h, h * 128:(h + 1) * 128] = 1.0
        sel4 = sel4.astype(BF)
        pc = np.zeros((BAND, D), f)
        pc[np.arange(BAND), g * BAND + np.arange(BAND)] = 0.0625
        in_maps.append({
            "xT": xTs[b].astype(BF),
            "xf8T": pack8(xTs[b].astype(F8NP)),
            "wqT": pack8(f8T(np.asarray(wq, f)[sl])),
            "wkT": pack8(f8T(np.asarray(wk, f)[sl])),
            "wvT": pack8(wv_aug.astype(F8NP)),
            "gwT": pack8(gw_aug.astype(F8NP)),
            "selT": sel4,
            "awc": np.ascontiguousarray(aw[4 * g:4 * g + 4].reshape(1, 4)),
            "cqT": pack8(bfT(np.asarray(causal_in_w, f)[0:D][sl] / 16.0)),
            "ckT": pack8(bfT(np.asarray(causal_in_w, f)[D:2 * D][sl])),
            "cvT": pack8(bfT(np.asarray(causal_in_w, f)[2 * D:][sl])),
            "cowT": pack2(np.ascontiguousarray(
                CAUSAL_ACTIVE * np.asarray(causal_out_w, f).T[sl]).astype(BF)),
            "pcT": pack2(pc.astype(F8NP)),
            "mqT": pack8(f8T(np.asarray(meta_in_w, f)[0:D][sl])),
            "mkT": pack8(f8T(np.asarray(meta_in_w, f)[D:2 * D][sl])),
            "mvT": pack8(f8T(np.asarray(meta_in_w, f)[2 * D:][sl])),
            "mowT": pack2(np.ascontiguousarray(MW * wfused[sl]).astype(BF)),
            "owT": pack2(np.ascontiguousarray(
                (1.0 - MW) * np.asarray(out_w, f).T[sl]).astype(BF)),
            "idT": np.eye(128, dtype=np.float32).astype(BF),
        })

    nc = _get_nc()
    res = run_bass_kernel_spmd(nc, in_maps, core_ids=list(range(NCORES)))

    bias_row = (np.asarray(out_b, f)
                + MW * (np.asarray(meta_out_b, f) @ np.asarray(out_w, f).T))
    out = np.empty((B, S, D), f)
    for b in range(B):
        acc = np.zeros((D, S), f)
        for g in range(G):
            acc += res.results[b * G + g]["outP"].astype(f)
        out[b] = acc.T + bias_row[None, :]
    return out



# revision 69
# speedup vs baseline: 1.0196x; 1.0196x over previous
"""Trainium2 8-core kernel for the AGI transformer block.

Sharding: 2-way data parallel over batch x 4-way tensor parallel over heads.
Core c: batch b=c//4, feature band g=c%4 (256 features = 4 main heads of 64 /
1 causal head of 256 / 1 meta head of 256).

Precision split by blend weight: the causal path (0.9) and the 0.85 final
out-proj stay bf16; the main path (0.1) and meta path (0.15) run fp8e4m3
with DoubleRow matmuls (two 128-deep contraction subtiles per instruction,
2 MACs/cycle), halving their PE stream time. fp8 operands are pre-scaled
(weights x16, ctx x8, psum casts x2^-k) to sit in e4m3's normal range; the
net scale is folded into the ACT Exp `scale` or the division multiply.

Per core (band slice G = [256g, 256g+256)):
  - main attention: 4 heads, sigmoid(gate+aw) modulation folded into q;
    rowsums via ones-column in the fp8 V (M=65); AV runs DoubleRow over
    j-tile pairs; softmax scale 1/8 folded into the Exp activation.
  - causal MHA head: hd=256 bf16, q pre-scaled 1/16; 0.9 blend folded into
    out-proj weight; main's ctx enters the same PSUM via a DoubleRow
    placement matmul (one-hot x 1/16, ctxm carries 1.6/rs).
  - blend combine: ReduceScatter(add) -> own band (0.85 term) + AllGather
    -> full ctx (meta). Softmax division uses DVE reciprocal_approx_fast
    (no ACT table switches) + a PE ones-matmul partition broadcast.
  - meta MHA head: hd=256 fp8 DoubleRow; 0.15*meta_out_w.T@out_w.T folded
    into one bf16 matrix.
  - final: outP = mowT.T@metaA + owT.T@band_ctx (partial; host sums 4).

Emission interleaves ACT-bound main attention with PE-bound causal attention
and meta projections so the TensorE stream stays dense.
"""

import os

import ml_dtypes
import numpy as np

DEBUG = os.environ.get("KDBG") == "1"

import concourse.mybir as mybir
import concourse.tile as tile
from concourse import bacc
from concourse.bass_utils import run_bass_kernel_spmd

F32 = mybir.dt.float32
BF16 = mybir.dt.bfloat16
F8 = mybir.dt.float8e4
AF = mybir.ActivationFunctionType
MUL = mybir.AluOpType.mult
DR = mybir.MatmulPerfMode.DoubleRow
DP = mybir.MatmulPerfMode.DoublePixel
BF = ml_dtypes.bfloat16
F8NP = ml_dtypes.float8_e4m3

B, S, D = 2, 2048, 1024
NCORES = 8
G = 4  # tensor-parallel group size
BAND = 256  # features per core
IC, NIC = 512, 4  # i-chunk (query) tiling
NJT = 16  # j tiles of 128
NPR = 8  # j-tile pairs per chunk
NKT = 8  # contraction tiles of 128 over D
CAUSAL_ACTIVE = 0.9
MW = ((0.9 - 0.8) / 0.2) * 0.3  # 0.15


def build_program():
    nc = bacc.Bacc("TRN2", target_bir_lowering=False, debug=False,
                   num_devices=NCORES)

    def din(name, shape, dt=BF16):
        return nc.dram_tensor(name, shape, dt, kind="ExternalInput").ap()

    # all weight/x tensors arrive HOST-PACKED as [128, nk, cols] (partition-
    # major) so each load is ONE DMA with a contiguous per-partition segment
    xT = din("xT", [D, S])
    xf8T = din("xf8T", [128, NKT, S], F8)
    wqT = din("wqT", [128, NKT, BAND], F8)
    wkT = din("wkT", [128, NKT, BAND], F8)
    wvT = din("wvT", [128, NKT, 320], F8)  # 4x(64 head cols + ones + pad 80)
    gwT = din("gwT", [128, NKT, 16], F8)  # 4 gate rows + zero pad
    selT = din("selT", [4, 512])  # 4 one-hot row-selector blocks [4,128]
    awc = nc.dram_tensor("awc", [1, 4], F32, kind="ExternalInput").ap()
    cqT = din("cqT", [128, NKT, BAND])
    ckT = din("ckT", [128, NKT, BAND])
    cvT = din("cvT", [128, NKT, BAND])
    cowT = din("cowT", [128, 2, D])
    pcT = din("pcT", [128, 2, D], F8)  # placement matrix (1/16 at own band)
    mqT = din("mqT", [128, NKT, BAND], F8)
    mkT = din("mkT", [128, NKT, BAND], F8)
    mvT = din("mvT", [128, NKT, BAND], F8)
    mowT = din("mowT", [128, 2, D])
    owT = din("owT", [128, 2, D])
    idT = din("idT", [128, 128])
    outP = nc.dram_tensor("outP", [D, S], BF16, kind="ExternalOutput").ap()
    dbg = {}
    if DEBUG:
        for nm, shape, dt in [
            ("d_mrow4", [4, S], BF16), ("d_kf8", [128, 2, S], F8),
            ("d_vsb", [128, NJT, 320], F8), ("d_ctxm", [128, 2, S], F8),
            ("d_cA", [128, 2, S], BF16),
            ("d_ctxF0", [128, NKT, IC], F8), ("d_mq", [128, 2, S], F8),
            ("d_mk", [128, 2, S], F8), ("d_mv", [128, NJT, BAND], F8),
            ("d_mA", [128, 2, S], BF16), ("d_bandC0", [128, 2, IC], BF16),
            ("d_qs", [128, 2, 2 * IC], F8),
        ]:
            dbg[nm] = nc.dram_tensor(nm, shape, dt,
                                     kind="ExternalOutput").ap()

    groups = [[0, 1, 2, 3], [4, 5, 6, 7]]

    with tile.TileContext(nc) as tc:
        with (
            tc.tile_pool(name="wts", bufs=1) as wts,
            tc.tile_pool(name="act", bufs=1) as actp,
            tc.tile_pool(name="small", bufs=1) as small,
            tc.tile_pool(name="work", bufs=3) as work,
            tc.tile_pool(name="stat", bufs=2) as statp,
            tc.tile_pool(name="psE", bufs=2, space="PSUM") as psE,
            tc.tile_pool(name="psA", bufs=4, space="PSUM") as psA,
            tc.tile_pool(name="dram", bufs=1, space="DRAM") as dram,
        ):
            def load_w(name, ap, cols, tag, dt=BF16, q=nc.sync):
                # host-packed [128, NKT, cols]: one contiguous-segment DMA
                t = wts.tile([128, NKT, cols], dt, name=name, tag=tag)
                q.dma_start(t[:, :, :], ap[:, :, :])
                return t

            def load_w2(name, ap, tag, dt=BF16, q=nc.sync):
                # host-packed [128, 2, 1024]: one DMA
                t = wts.tile([128, 2, D], dt, name=name, tag=tag)
                q.dma_start(t[:, :, :], ap[:, :, :])
                return t

            # warmup barrier: absorbs inter-core launch skew while the
            # startup DMAs stream, so later collectives start aligned
            warm_sb = small.tile([1, 4], F32)
            nc.vector.memset(warm_sb[:, :], 0.0)
            warmA = dram.tile([1, 4], F32, name="warmA", tag="warmA")
            warmB = dram.tile([1, 4], F32, name="warmB", tag="warmB")
            nc.sync.dma_start(warmA[:, :], warm_sb[:, :])
            nc.gpsimd.collective_compute(
                "AllReduce", mybir.AluOpType.add, replica_groups=groups,
                ins=[warmA[:, :].opt()], outs=[warmB[:, :].opt()])

            gwf = load_w("gwf", gwT, 16, "gw", F8)
            aw_sb = small.tile([4, 1], F32)
            nc.sync.dma_start(aw_sb[:, :], awc.rearrange("a b -> b a"))
            sel_sb = small.tile([4, 512], BF16)
            nc.sync.dma_start(sel_sb[:, :], selT[:, :])

            # fp8 x in DoubleRow layout, split in two tiles (kt 0-3 / 4-7)
            # whose slots are later reused by the meta ctx fp8 chunks 2/3;
            # one DMA per tile (startup is dispatch-latency bound)
            xf8 = [actp.tile([128, 4, S], F8, name=f"xf8{i}", tag=f"xf8{i}")
                   for i in range(2)]
            # split across two queues so the transfers run in parallel
            nc.sync.dma_start(xf8[0][:, 0:2, :], xf8T[:, 0:2, :])
            nc.sync.dma_start(xf8[0][:, 2:4, :], xf8T[:, 2:4, :])
            nc.gpsimd.dma_start(xf8[1][:, :, :], xf8T[:, 4:8, :])

            def xf8_sl(kp, c0, cw):  # kt-pair kp as [128, 2, cw] slice
                t, r = xf8[kp // 2], (kp % 2) * 2
                return t[:, r:r + 2, c0:c0 + cw]

            wqf = load_w("wqf", wqT, BAND, "wq", F8)
            wkf = load_w("wkf", wkT, BAND, "wk", F8)
            wvf = load_w("wvf", wvT, 320, "wv", F8)

            # bf16 x per-kt tiles (causal path) on the ACT hwdge queue so
            # they don't starve the startup-critical fp8 loads; tags pair
            # them with later-stage tiles so the SBUF slots time-share
            xtags = ["ctxC0", "ctxC1", "ctxC2", "ctxC3",
                     "qT2", "kT2", "vA2", "bandC0"]
            xT_t = []
            for kt in range(NKT):
                t = actp.tile([128, S], BF16, name=f"xTt{kt}", tag=xtags[kt])
                nc.scalar.dma_start(t[:, :], xT[kt * 128:(kt + 1) * 128, :])
                xT_t.append(t)

            # causal weights ride the gpsimd queue (idle until the first
            # collective) to keep the sync queue for startup-critical loads
            cq_sb = load_w("cq_sb", cqT, BAND, "cq", q=nc.gpsimd)
            ck_sb = load_w("ck_sb", ckT, BAND, "ck", q=nc.gpsimd)
            cv_sb = load_w("cv_sb", cvT, BAND, "cv", q=nc.gpsimd)
            ones_sb = small.tile([128, 1], BF16)
            nc.vector.memset(ones_sb[:, :], 1.0)
            onesrow = small.tile([1, 128], BF16)
            nc.vector.memset(onesrow[:, :], 1.0)
            onesrow3 = small.tile([65, 128], BF16)
            nc.vector.memset(onesrow3[:, :], 1.0)
            ones_pair = small.tile([128, 2, 16], F8)
            nc.vector.memset(ones_pair[:, :, :], 1.0)

            def pe_keepwarm(n=8):
                # dependency-free LDWEIGHTS burst across division waits
                for _ in range(n):
                    nc.tensor.ldweights(kf8_sb[:, 0, 0:128])

            # ---------- projections ----------
            qT_sb = actp.tile([128, 2, S], BF16, tag="qT")
            kf8_sb = actp.tile([128, 2, S], F8, tag="kT")

            def proj_chunk_dr(w_f8, ot, icc):
                ps = psA.tile([128, IC], F32, tag="acc")
                for kp in range(4):
                    nc.tensor.matmul(
                        ps[:, :],
                        w_f8[:, 2 * kp:2 * kp + 2, ot * 128:(ot + 1) * 128],
                        xf8_sl(kp, icc * IC, IC),
                        start=(kp == 0), stop=(kp == 3), perf_mode=DR)
                return ps

            def proj_chunk(dst, w_sb, src_t, ot, icc):  # bf16 (causal)
                ps = psA.tile([128, IC], F32, tag="acc")
                for kt in range(NKT):
                    nc.tensor.matmul(
                        ps[:, :],
                        w_sb[:, kt, ot * 128:(ot + 1) * 128],
                        src_t[kt][:, icc * IC:(icc + 1) * IC],
                        start=(kt == 0), stop=(kt == NKT - 1))
                nc.scalar.copy(dst[:, ot, icc * IC:(icc + 1) * IC],
                               ps[:, :])

            def proj_T(dst, w_sb, src_t):
                for ot in range(2):
                    for icc in range(4):
                        proj_chunk(dst, w_sb, src_t, ot, icc)

            # gate matmuls + sigmoid for ALL chunks now (keeps the sigmoid
            # table switch out of the attention blocks)
            mrow4 = small.tile([4, S], BF16)
            for icc in range(4):
                i0 = icc * IC
                g_ps2 = psE.tile([128, 2, IC], F32, tag="eps2")
                g_ps = g_ps2[0:16, 0, :]
                for kp in range(4):
                    nc.tensor.matmul(g_ps[:, :],
                                     gwf[:, 2 * kp:2 * kp + 2, 0:16],
                                     xf8_sl(kp, i0, IC),
                                     start=(kp == 0), stop=(kp == 3),
                                     perf_mode=DR)
                nc.scalar.activation(mrow4[:, i0:i0 + IC], g_ps[0:4, :],
                                     AF.Sigmoid, bias=aw_sb[:, 0:1],
                                     scale=1.0 / 16)

            qs_sb = actp.tile([128, 2, 2 * IC], F8, tag="qs")  # 2-chunk ring

            def qmod(h, icc):
                # broadcast row h of mrow4 to 128 partitions via a K=4 matmul
                # against a one-hot selector, then fold sigma into fp8 q
                rh, oh = (h % 2) * 64, h // 2
                i0 = icc * IC
                pb2 = psE.tile([128, 2, IC], F32, tag="eps2")
                nc.tensor.matmul(pb2[:, 0, :],
                                 sel_sb[0:4, h * 128:(h + 1) * 128],
                                 mrow4[0:4, i0:i0 + IC])
                r0 = (icc % 2) * IC
                nc.vector.tensor_mul(qs_sb[rh:rh + 64, oh, r0:r0 + IC],
                                     qT_sb[rh:rh + 64, oh, i0:i0 + IC],
                                     pb2[rh:rh + 64, 0, :])

            def qproj_steps(icc):
                """q projection/modulation filler steps for chunk icc"""
                steps = []
                for ot in range(2):
                    def sq(ot=ot, icc=icc):
                        ps = proj_chunk_dr(wqf, ot, icc)
                        nc.vector.tensor_scalar_mul(
                            qT_sb[:, ot, icc * IC:(icc + 1) * IC], ps[:, :],
                            1.0 / 16)
                        qmod(2 * ot, icc)
                        qmod(2 * ot + 1, icc)
                    steps.append(sq)
                return steps

            # stage B: only the first-processed chunk of q; full k/v and the
            # whole causal path (keeps xT readers out of the chunk pipeline
            # so the deferred ctxC read-backs never park the sync queue)
            cqT_sb = actp.tile([128, 2, S], BF16, tag="cqT")
            for st_ in qproj_steps(3):
                st_()
            proj_T(cqT_sb, cq_sb, xT_t)
            for ot in range(2):
                for icc in range(4):
                    ps = proj_chunk_dr(wkf, ot, icc)
                    nc.scalar.mul(kf8_sb[:, ot, icc * IC:(icc + 1) * IC],
                                  ps[:, :], 1.0 / 16)

            # v natural layout [2048 j, 320]: head h at cols 80h..80h+63,
            # ones at 80h+64 (written post-copy)
            v_sb = actp.tile([128, NJT, 320], F8, tag="vA")
            for st in range(NJT):
                ps = psA.tile([128, 320], F32, tag="acc")
                for kp in range(4):
                    nc.tensor.matmul(ps[:, :],
                                     xf8_sl(kp, st * 128, 128),
                                     wvf[:, 2 * kp:2 * kp + 2, :],
                                     start=(kp == 0), stop=(kp == 3),
                                     perf_mode=DR)
                nc.scalar.mul(v_sb[:, st, :], ps[:, :], 1.0 / 16)
                nc.vector.memset(v_sb[:, st, 64:320:80], 1.0)

            ckT_sb = actp.tile([128, 2, S], BF16, tag="ckT")
            proj_T(ckT_sb, ck_sb, xT_t)

            cv_nat = actp.tile([128, NJT, BAND], BF16, tag="cvN")
            for st in range(NJT):
                ps = psA.tile([128, BAND], F32, tag="acc")
                for kt in range(NKT):
                    nc.tensor.matmul(ps[:, :],
                                     xT_t[kt][:, st * 128:(st + 1) * 128],
                                     cv_sb[:, kt, :],
                                     start=(kt == 0), stop=(kt == NKT - 1))
                nc.scalar.copy(cv_nat[:, st, :], ps[:, :])

            # late-stage weights: load now (off the startup critical path;
            # slots of wq/wk/wv just freed); gpsimd queue keeps sync free
            mqf = load_w("mqf", mqT, BAND, "wq", F8, q=nc.gpsimd)
            mkf = load_w("mkf", mkT, BAND, "wk", F8, q=nc.gpsimd)
            mvf = load_w("mvf", mvT, BAND, "wv", F8, q=nc.gpsimd)
            cow_sb = load_w2("cow_sb", cowT, "cow", q=nc.gpsimd)
            pcf = load_w2("pcf", pcT, "pc", F8, q=nc.gpsimd)
            mow_sb = load_w2("mow_sb", mowT, "mow", q=nc.gpsimd)
            id_sb = small.tile([128, 128], BF16)
            nc.gpsimd.dma_start(id_sb[:, :], idT[:, :])
            ow_sb = load_w2("ow_sb", owT, "ow", q=nc.gpsimd)

            # ---------- chunked tiles ----------
            ctxm_sb = actp.tile([128, 2, S], F8, tag="ctxm")  # 1.6*main ctx
            cA_sb = actp.tile([128, 2, S], BF16, tag="cA")
            # fp8 ctx (x8) for the meta projections, straight from the fp8
            # AllGather; chunk 2 (last processed) reuses an xf8 slot (dead
            # after the final q projections)
            ctxF = [actp.tile([128, NKT, IC], F8, name=f"ctxF{i}",
                              tag=("ctxF0", "ctxF1", "xf80", "ctxF3")[i])
                    for i in range(NIC)]
            bandC = [actp.tile([128, 2, IC], BF16, name=f"bandC{i}",
                               tag=f"bandC{i}") for i in range(NIC)]
            mqT_f8 = actp.tile([128, 2, S], F8, tag="qT2")
            mkT_f8 = actp.tile([128, 2, S], F8, tag="kT2")
            mv_nat = actp.tile([128, NJT, BAND], F8, tag="vA2")

            # chunks are processed in ORDER so the last collective (chunk 2)
            # is needed only by the tail of the meta phase
            ORDER = [3, 0, 1, 2]
            LASTC = ORDER[-1]
            arB, rsO, agI, agF = [], [], [], []
            for icc in range(NIC):
                nh = 1
                arB.append([dram.tile([D, IC // nh], BF16,
                                      name=f"arB{icc}_{hh}", tag=f"arB{icc}{hh}")
                            for hh in range(nh)])
                rsO.append([dram.tile([BAND, IC // nh], BF16,
                                      name=f"rsO{icc}_{hh}", tag=f"rsO{icc}{hh}")
                            for hh in range(nh)])
                agI.append([dram.tile([128, 2 * (IC // nh)], F8,
                                      name=f"agI{icc}_{hh}", tag=f"agI{icc}{hh}")
                            for hh in range(nh)])
                agF.append([dram.tile([4 * 128, 2 * (IC // nh)], F8,
                                      name=f"agF{icc}_{hh}", tag=f"agF{icc}{hh}")
                            for hh in range(nh)])
            def agchain(icc, hh, c0, hw_):
                # fp8 cast + AllGather + readbacks; the WHOLE chain rides
                # the gpsimd queue so a late RS parks only its own chain
                bcf = work.tile([128, 2, hw_], F8, tag="bcf", bufs=2,
                                name=f"bcf{icc}{hh}")
                nc.gpsimd.tensor_scalar_mul(
                    bcf[:, :, :], bandC[icc][:, 0:2, c0:c0 + hw_], 8.0)
                nc.gpsimd.dma_start(
                    agI[icc][hh][:, :],
                    bcf[:, :, :].rearrange("p a c -> p (a c)"))
                nc.gpsimd.collective_compute(
                    "AllGather", mybir.AluOpType.bypass,
                    replica_groups=groups,
                    ins=[agI[icc][hh][:, :].opt()],
                    outs=[agF[icc][hh][:, :].opt()])
                for g2 in range(4):
                    nc.gpsimd.dma_start(
                        ctxF[icc][:, 2 * g2:2 * g2 + 2, c0:c0 + hw_],
                        agF[icc][hh][g2 * 128:(g2 + 1) * 128, :]
                        .rearrange("p (a c) -> p a c", a=2))

            def div_prep(specs, i0):
                """softmax divisions, split for PE continuity: the DVE
                fast-reciprocal chain is emitted NOW (it only needs the
                rowsums); the PE ones-matmul broadcast + DVE multiplies are
                returned as filler closures to weave into later PE work.
                spec: ("head", h, acc64_ap, rs_ap)
                   or ("wide", dst_sb, a1, a2, rs_ap, sc)"""
                # batched reciprocal: relocate the rowsums to partitions
                # 0/32/64 of ONE tile (legal PE moving-base positions), then
                # a single reciprocal + cast serve every spec — DVE time
                # scales with free size, not partitions, so this is ~2x
                # lower latency for the chain the PE broadcasts wait on
                rs3 = statp.tile([65, IC], F32, tag="rs0", bufs=1)
                for i_, sp in enumerate(specs):
                    rs_ap = sp[3] if sp[0] == "head" else sp[4]
                    nc.vector.tensor_copy(rs3[32 * i_:32 * i_ + 1, :], rs_ap)
                rcp3 = statp.tile([65, IC], F32, tag="lnr", bufs=2)
                nc.vector.reciprocal_approx_fast(rcp3[:, :], rs3[:, :])
                rcpb3 = statp.tile([65, IC], BF16, tag="rcp", bufs=3)
                nc.vector.tensor_copy(rcpb3[:, :], rcp3[:, :])
                rcps = [rcpb3[32 * i_:32 * i_ + 1, :]
                        for i_ in range(len(specs))]

                def one(sp, rcpb, i0=i0):
                    n = 64 if sp[0] == "head" else 128
                    bp = rcpb.base_partition()
                    pb_ps = psE.tile([128, 2, IC], F32, tag="eps2")
                    nc.tensor.matmul(pb_ps[:, 0, :],
                                     onesrow3[bp:bp + 1, :], rcpb)
                    pb = work.tile([n, IC], BF16,
                                   tag="pbm" if n == 64 else "pb2", bufs=3)
                    nc.vector.tensor_copy(pb[:, :], pb_ps[0:n, 0, :])
                    if sp[0] == "head":
                        h, acc64 = sp[1], sp[2]
                        rh, oh = (h % 2) * 64, h // 2
                        nc.vector.scalar_tensor_tensor(
                            ctxm_sb[rh:rh + 64, oh, i0:i0 + IC],
                            acc64, 1.6, pb[:, :], MUL, MUL)
                    else:
                        dst_sb, a1, a2, sc = sp[1], sp[2], sp[3], sp[5]
                        nc.vector.scalar_tensor_tensor(
                            dst_sb[:, 0, i0:i0 + IC], a1[:, :], sc,
                            pb[:, :], MUL, MUL)
                        nc.vector.scalar_tensor_tensor(
                            dst_sb[:, 1, i0:i0 + IC], a2[:, :], sc,
                            pb[:, :], MUL, MUL)
                return [lambda sp=sp, rcpb=rcpb: one(sp, rcpb)
                        for sp, rcpb in zip(specs, rcps)]

            def main_pair_step(p, t, i0, accs, mid=None):
                """jt pair (2t, 2t+1) for main heads (2p, 2p+1): per head one
                2-bank score psum (both jt) -> ONE merged Exp -> DoubleRow AV
                (M=65: v cols + ones-column rowsum at row 64). DR dsts must
                sit at partition 0, hence one [65,IC] bank per head.
                mid() (if given) emits between the scores and the first AV —
                the hook point for deferred division broadcasts that must
                precede the accumulator-claiming AV in the PE FIFO."""
                oh = p
                r0 = (i0 // IC % 2) * IC
                esbs = [work.tile([128, 2, IC], F8, tag="esb2", bufs=4,
                                  name=f"esb{p}{t}{i0}{hh}")
                        for hh in range(2)]
                for hh in range(2):
                    rh = hh * 64
                    eps2 = psE.tile([128, 2, IC], F32, tag="eps2")
                    for dj in range(2):
                        jt = 2 * t + dj
                        nc.tensor.matmul(
                            eps2[:, dj, :],
                            kf8_sb[rh:rh + 64, oh, jt * 128:(jt + 1) * 128],
                            qs_sb[rh:rh + 64, oh, r0:r0 + IC])
                    nc.scalar.activation(esbs[hh][:, 0:2, :], eps2[:, 0:2, :],
                                         AF.Exp, scale=0.125)
                if mid is not None:
                    mid()
                for hh in range(2):
                    h = 2 * p + hh
                    nc.tensor.matmul(
                        accs[hh][:, :],
                        v_sb[:, 2 * t:2 * t + 2, 80 * h:80 * h + 65],
                        esbs[hh][:, 0:2, :],
                        start=(t == 0), stop=(t == NPR - 1), perf_mode=DR)

            def causal_attn_step(t, i0, a1, a2, crs_acc):
                """jt pair (2t, 2t+1) of the bf16 hd-256 causal attention:
                one 2-bank score psum for both jt -> ONE merged Exp; the
                rowsum partials land back in the (exp-consumed) score psum
                and accumulate into SBUF crs_acc on the DVE"""
                eps2 = psE.tile([128, 2, IC], F32, tag="eps2")
                for dj in range(2):
                    jt = 2 * t + dj
                    for dkt in range(2):
                        nc.tensor.matmul(
                            eps2[:, dj, :],
                            ckT_sb[:, dkt, jt * 128:(jt + 1) * 128],
                            cqT_sb[:, dkt, i0:i0 + IC],
                            start=(dkt == 0), stop=(dkt == 1))
                esb = work.tile([128, 2, IC], BF16, tag="esb", bufs=4)
                nc.scalar.activation(esb[:, 0:2, :], eps2[:, 0:2, :], AF.Exp)
                for dj in range(2):
                    jt = 2 * t + dj
                    st_, sp_ = (jt == 0), (jt == NJT - 1)
                    nc.tensor.matmul(a1[:, :], cv_nat[:, jt, 0:128],
                                     esb[:, dj, :], start=st_, stop=sp_)
                    nc.tensor.matmul(a2[:, :], cv_nat[:, jt, 128:256],
                                     esb[:, dj, :], start=st_, stop=sp_)
                    nc.tensor.matmul(eps2[0:1, 0, :], ones_sb[:, 0:1],
                                     esb[:, dj, :],
                                     start=(dj == 0), stop=(dj == 1))
                nc.vector.tensor_add(crs_acc[0:1, :], crs_acc[0:1, :],
                                     eps2[0:1, 0, :])

            def meta_attn_step(t, i0, a1, a2, rs, st_, sp_, mid=None,
                               st_rs=None):
                if st_rs is None:
                    st_rs = st_
                """jt pair (2t, 2t+1) of the fp8 DoubleRow meta attention;
                one 2-bank score psum for both jt -> ONE merged Exp"""
                esbm = work.tile([128, 2, IC], F8, tag="esbm", bufs=3)
                eps2 = psE.tile([128, 2, IC], F32, tag="eps2")
                for dj in range(2):
                    jt = 2 * t + dj
                    nc.tensor.matmul(
                        eps2[:, dj, :],
                        mkT_f8[:, 0:2, jt * 128:(jt + 1) * 128],
                        mqT_f8[:, 0:2, i0:i0 + IC], perf_mode=DR)
                nc.scalar.activation(esbm[:, 0:2, :], eps2[:, 0:2, :],
                                     AF.Exp, scale=1.0 / 256)
                if mid is not None:
                    mid()
                nc.tensor.matmul(a1[:, :], mv_nat[:, 2 * t:2 * t + 2, 0:128],
                                 esbm[:, 0:2, :], start=st_, stop=sp_,
                                 perf_mode=DR)
                nc.tensor.matmul(a2[:, :], mv_nat[:, 2 * t:2 * t + 2, 128:256],
                                 esbm[:, 0:2, :], start=st_, stop=sp_,
                                 perf_mode=DR)
                nc.tensor.matmul(rs[:, :], ones_pair[:, 0:2, 0:1],
                                 esbm[:, 0:2, :], start=st_rs, stop=sp_,
                                 perf_mode=DR)

            def metaproj_steps(icc):
                """closures emitting chunk icc's meta projections (fp8 DR)"""
                i0 = icc * IC
                steps = []


                def projstep(w_f8, dst, ot, i0=i0, icc=icc):
                    ps = psA.tile([128, IC], F32, tag="acc")
                    for kp in range(4):
                        nc.tensor.matmul(
                            ps[:, :],
                            w_f8[:, 2 * kp:2 * kp + 2,
                                 ot * 128:(ot + 1) * 128],
                            ctxF[icc][:, 2 * kp:2 * kp + 2, :],
                            start=(kp == 0), stop=(kp == 3), perf_mode=DR)
                    nc.vector.tensor_scalar_mul(dst[:, ot, i0:i0 + IC],
                                                ps[:, :], 2.0 ** -5)

                def vstep(st4, icc=icc):
                    st = icc * 4 + st4
                    ps = psA.tile([128, BAND], F32, tag="acc")
                    for kp in range(4):
                        nc.tensor.matmul(
                            ps[:, :],
                            ctxF[icc][:, 2 * kp:2 * kp + 2,
                                      st4 * 128:(st4 + 1) * 128],
                            mvf[:, 2 * kp:2 * kp + 2, 0:BAND],
                            start=(kp == 0), stop=(kp == 3), perf_mode=DR)
                    nc.vector.tensor_scalar_mul(mv_nat[:, st, :], ps[:, :],
                                                2.0 ** -5)

                for ot in range(2):
                    steps.append(lambda ot=ot: projstep(mqf, mqT_f8, ot))
                for ot in range(2):
                    steps.append(lambda ot=ot: projstep(mkf, mkT_f8, ot))
                for st4 in range(4):
                    steps.append(lambda st4=st4: vstep(st4))
                return steps

            # ---------- per-chunk pipeline ----------
            # the fp8 cast + AllGather chain of chunk X is deferred into
            # chunk X+1's phase 2, when X's ReduceScatter is long done —
            # the DVE cast then never parks the strict-FIFO Vector queue
            pending_ag = []
            for oi, icc in enumerate(ORDER):
                i0 = icc * IC
                # phase 1: main heads (0,1) interleaved with causal attention
                accA = [psA.tile([65, IC], F32, tag="acc",
                                 name=f"accA{icc}{i}") for i in range(2)]
                ca1 = psA.tile([128, IC], F32, tag="acc")
                ca2 = psA.tile([128, IC], F32, tag="acc")
                crs_acc = statp.tile([1, IC], F32, tag="crsa", bufs=1,
                                     name=f"crsa{icc}")
                nc.vector.memset(crs_acc[:, :], 0.0)
                for t in range(NPR):
                    main_pair_step(0, t, i0, accA)
                    causal_attn_step(t, i0, ca1, ca2, crs_acc)
                # DVE reciprocal chain starts now; PE broadcasts deferred
                div1 = div_prep(
                    [("head", 0, accA[0][0:64, :], accA[0][64:65, :]),
                     ("head", 1, accA[1][0:64, :], accA[1][64:65, :]),
                     ("wide", cA_sb, ca1, ca2, crs_acc[0:1, :], 1.0)], i0)

                # phase 2: main heads (2,3) interleaved with filler PE work:
                # next chunk's q/cq projections + older chunk's meta projs
                accB = [psA.tile([65, IC], F32, tag="acc",
                                 name=f"accB{icc}{i}") for i in range(2)]
                fillers = []
                if oi + 1 < NIC:
                    fillers += qproj_steps(ORDER[oi + 1])
                while pending_ag:
                    pending_ag.pop(0)()
                nfront = len(fillers)
                done = 0
                for t in range(NPR):
                    # pair 0 emits the phase-1 division broadcasts between
                    # its scores and its accumulator-claiming AVs: the AV's
                    # WAR on the accA slots resolves through the divisions'
                    # DVE multiplies, which must precede it in the PE FIFO
                    mid = (lambda: [st_() for st_ in div1]) if t == 0 else None
                    main_pair_step(1, t, i0, accB, mid=mid)
                    # front-load fillers (drain by pair 3): the next chunk's
                    # phase-1 scores need the qmod outputs, and phase 2's
                    # tail is ACT-bound anyway so late-phase PE idle is free
                    want = min(nfront, (t + 1) * ((nfront + 2) // 3))
                    while done < want:
                        fillers[done]()
                        done += 1
                div2 = div_prep(
                    [("head", 2, accB[0][0:64, :], accB[0][64:65, :]),
                     ("head", 3, accB[1][0:64, :], accB[1][64:65, :])], i0)

                # causal out-proj + DoubleRow main placement -> arB chunk.
                # cow-only groups open first (they need only cA); the pc
                # close (needs ctxm h2/h3) trails by one ot so the phase-2
                # division broadcasts hide behind cow matmuls
                def close_ot(ot, ps):
                    nc.tensor.matmul(
                        ps[:, :],
                        pcf[:, 0:2, ot * 128:(ot + 1) * 128],
                        ctxm_sb[:, 0:2, i0:i0 + IC],
                        start=False, stop=True, perf_mode=DR)
                    ob = work.tile([128, IC], BF16, tag="obA", bufs=2)
                    nc.vector.tensor_copy(ob[:, :], ps[:, :])
                    nh = len(arB[icc])
                    hw_ = IC // nh
                    for hh in range(nh):
                        nc.sync.dma_start(
                            arB[icc][hh][ot * 128:(ot + 1) * 128, :],
                            ob[:, hh * hw_:(hh + 1) * hw_])

                if True:
                    opens = []
                    for ot in range(8):
                        ps = psA.tile([128, IC], F32, tag="acc")
                        for ft in range(2):
                            nc.tensor.matmul(
                                ps[:, :],
                                cow_sb[:, ft, ot * 128:(ot + 1) * 128],
                                cA_sb[:, ft, i0:i0 + IC],
                                start=(ft == 0), stop=False)
                        opens.append((ot, ps))
                        if ot < 2:
                            div2[ot]()
                        else:
                            close_ot(*opens.pop(0))
                        if done < len(fillers) and ot % 2 == 1:
                            fillers[done]()
                            done += 1
                    while opens:
                        close_ot(*opens.pop(0))
                else:
                    # LAST chunk: column-split outproj so half 0's stores
                    # finish early and its RS launches mid-outproj; the two
                    # half-chains (RS -> cast -> AG) then pipeline, roughly
                    # halving the exposed latency at the meta seam
                    def close_half(ot, ps, hf):
                        c0h = i0 + hf * 256
                        nc.tensor.matmul(
                            ps[:, :],
                            pcf[:, 0:2, ot * 128:(ot + 1) * 128],
                            ctxm_sb[:, 0:2, c0h:c0h + 256],
                            start=False, stop=True, perf_mode=DR)
                        ob = work.tile([128, 256], BF16, tag="obA", bufs=2)
                        nc.vector.tensor_copy(ob[:, :], ps[:, :])
                        nc.sync.dma_start(
                            arB[icc][hf][ot * 128:(ot + 1) * 128, :],
                            ob[:, :])
                    for hf in range(2):
                        c0h = i0 + hf * 256
                        opens = []
                        for ot in range(8):
                            ps = psA.tile([128, 256], F32, tag="acc")
                            for ft in range(2):
                                nc.tensor.matmul(
                                    ps[:, :],
                                    cow_sb[:, ft, ot * 128:(ot + 1) * 128],
                                    cA_sb[:, ft, c0h:c0h + 256],
                                    start=(ft == 0), stop=False)
                            opens.append((ot, ps))
                            if hf == 0 and ot < 2:
                                div2[ot]()
                            elif len(opens) > 1:
                                close_half(*opens.pop(0), hf)
                        while opens:
                            close_half(*opens.pop(0), hf)
                        nc.gpsimd.collective_compute(
                            "ReduceScatter", mybir.AluOpType.add,
                            replica_groups=groups,
                            ins=[arB[icc][hf][:, :].opt()],
                            outs=[rsO[icc][hf][:, :].opt()])
                        for kt in range(2):
                            nc.gpsimd.dma_start(
                                bandC[icc][:, kt, hf * 256:hf * 256 + 256],
                                rsO[icc][hf][kt * 128:(kt + 1) * 128, :])
                        pending_ag.append(
                            lambda icc=icc, hf=hf: agchain(icc, hf,
                                                           hf * 256, 256))

                # blend combine: ReduceScatter (bf16 own band -> bandC) then
                # AllGather of the fp8-cast band (half the bytes -> ctxF).
                # The last chunk is split in half for a shorter latency tail.
                # Read-backs ride the gpsimd queue: parking it on a
                # collective's completion is free (the CC engine is serial
                # anyway) and the sync queue stays park-free for the stores.
                if True:
                    nc.gpsimd.collective_compute(
                        "ReduceScatter", mybir.AluOpType.add,
                        replica_groups=groups,
                        ins=[arB[icc][0][:, :].opt()],
                        outs=[rsO[icc][0][:, :].opt()])
                    for kt in range(2):
                        nc.gpsimd.dma_start(
                            bandC[icc][:, kt, :],
                            rsO[icc][0][kt * 128:(kt + 1) * 128, :])
                    pending_ag.append(
                        lambda icc=icc: agchain(icc, 0, 0, IC))

            # ALL meta projections emit after the chunk pipeline: emitting
            # them as phase-2 fillers head-blocks the PE FIFO on the
            # collective-derived ctxF whenever cores drift apart. Chunk 2's
            # are emitted inside the first meta attention chunk below.
            while pending_ag:
                pending_ag.pop(0)()
            for c_ in (ORDER[0], ORDER[1], ORDER[2]):
                for st in metaproj_steps(c_):
                    st()

            # ---------- meta attention + final out-proj ----------
            def final_steps(icc):
                i0 = icc * IC
                steps = []

                def fstep(ot, icc=icc, i0=i0):
                    ps = psA.tile([128, IC], F32, tag="acc")
                    for ft in range(2):
                        nc.tensor.matmul(
                            ps[:, :],
                            mow_sb[:, ft, ot * 128:(ot + 1) * 128],
                            mA_sb[:, ft, i0:i0 + IC],
                            start=(ft == 0), stop=False)
                    for ft in range(2):
                        nc.tensor.matmul(
                            ps[:, :],
                            ow_sb[:, ft, ot * 128:(ot + 1) * 128],
                            bandC[icc][:, ft, :],
                            start=False, stop=(ft == 1))
                    ob = work.tile([128, IC], BF16, tag="obF", bufs=2)
                    nc.vector.tensor_copy(ob[:, :], ps[:, :])
                    nc.sync.dma_start(
                        outP[ot * 128:(ot + 1) * 128, i0:i0 + IC], ob[:, :])
                for ot in range(8):
                    steps.append(lambda ot=ot: fstep(ot))
                return steps

            # j-pair order matches ctx availability order (chunk 2 last)
            PAIRS = [6, 7, 0, 1, 2, 3, 4, 5]
            mA_sb = actp.tile([128, 2, S], BF16, tag="cqT")  # reuse slot

            # PASS A — chain-free coverage: for q-chunks 3,0,1 run the six
            # j-pairs that avoid j-chunk 2, partial-close, and stage the
            # partial sums to SBUF (bf16; 0.15-weighted meta tolerates it).
            # This work fills the window while chunk 2's RS/AG chain flies.
            stageA = {}
            for c_ in (ORDER[0], ORDER[1], ORDER[2]):
                i0 = c_ * IC
                a1 = psA.tile([128, IC], F32, tag="acc")
                a2 = psA.tile([128, IC], F32, tag="acc")
                rs = psA.tile([1, IC], F32, tag="acc", name="mrs")
                for pi in range(6):
                    meta_attn_step(PAIRS[pi], i0, a1, a2, rs,
                                   pi == 0, pi == 5)
                sg = work.tile([128, 2, IC], BF16, tag="esb", bufs=4,
                               name=f"mstg{c_}")
                sgr = statp.tile([1, IC], F32, tag="mstgr", bufs=3,
                                 name=f"mstgr{c_}")
                nc.vector.tensor_copy(sg[:, 0, :], a1[:, :])
                nc.vector.tensor_copy(sg[:, 1, :], a2[:, :])
                nc.vector.tensor_copy(sgr[:, :], rs[:, :])
                stageA[c_] = (sg, sgr)

            # chunk 2's meta projections — first consumer of the collective
            for st in metaproj_steps(LASTC):
                st()

            # PASS B — per q-chunk: reinject staged partials (identity
            # matmul re-seeds the psum), finish the j-chunk-2 pairs, divide,
            # and interleave the previous chunk's final out-proj steps
            # q-chunk 2 (the only one needing a full 8-pair pass) goes
            # FIRST: it is ready the moment metaproj(2) lands, and the tail
            # then ends on a 2-pair reinjected chunk instead of 8 pairs
            PB = [LASTC] + [c for c in ORDER if c != LASTC]
            pend_div = []
            for mi, icc in enumerate(PB):
                i0 = icc * IC
                fsteps = final_steps(PB[mi - 1]) if mi > 0 else []
                for st_ in pend_div:
                    st_()
                pend_div = []
                a1 = psA.tile([128, IC], F32, tag="acc")
                a2 = psA.tile([128, IC], F32, tag="acc")
                rs = psA.tile([1, IC], F32, tag="acc", name="mrs")
                if icc == LASTC:
                    for pi in range(NPR):
                        meta_attn_step(PAIRS[pi], i0, a1, a2, rs,
                                       pi == 0, pi == NPR - 1)
                        if fsteps and pi < len(fsteps):
                            fsteps[pi]()
                    fsteps = fsteps[NPR:]
                    rs_ap = rs[0:1, :]
                else:
                    sg, sgr = stageA[icc]
                    nc.tensor.matmul(a1[:, :], id_sb[:, :], sg[:, 0, :],
                                     start=True, stop=False)
                    nc.tensor.matmul(a2[:, :], id_sb[:, :], sg[:, 1, :],
                                     start=True, stop=False)
                    if fsteps:
                        fsteps[0]()
                    for pi in range(6, NPR):
                        meta_attn_step(PAIRS[pi], i0, a1, a2, rs,
                                       False, pi == NPR - 1,
                                       st_rs=(pi == 6))
                        if len(fsteps) > pi - 5:
                            fsteps[pi - 5]()
                    fsteps = fsteps[3:]
                    # merged rowsum: staged partial + j-chunk-2 psum part
                    rs_m = statp.tile([1, IC], F32, tag="mrsm", bufs=1,
                                      name=f"mrsm{icc}")
                    nc.vector.tensor_add(rs_m[0:1, :], sgr[0:1, :],
                                         rs[0:1, :])
                    rs_ap = rs_m[0:1, :]
                for fs in fsteps:
                    fs()
                pend_div = div_prep([("wide", mA_sb, a1, a2, rs_ap, 0.25)],
                                    i0)

            for st_ in pend_div:
                st_()
            for st in final_steps(PB[-1]):
                st()

            if DEBUG:
                for nm, t in [
                    ("d_mrow4", mrow4), ("d_kf8", kf8_sb), ("d_vsb", v_sb),
                    ("d_ctxm", ctxm_sb), ("d_cA", cA_sb),
                    ("d_ctxF0", ctxF[0]),
                    ("d_mq", mqT_f8), ("d_mk", mkT_f8), ("d_mv", mv_nat),
                    ("d_mA", mA_sb), ("d_bandC0", bandC[0]),
                    ("d_qs", qs_sb),
                ]:
                    ap = dbg[nm]
                    if len(t.shape) == 2:
                        nc.sync.dma_start(ap[:, :], t[:, :])
                    else:
                        nc.sync.dma_start(ap[:, :, :], t[:, :, :])

    nc.compile()
    return nc


_NC = None


def _get_nc():
    global _NC
    if _NC is None:
        _NC = build_program()
    return _NC


def kernel(hidden_states, consciousness_vector, wq, bq, wk, bk, wv, bv,
           gate_w, gate_b, aw_w, aw_b,
           causal_in_w, causal_in_b, causal_out_w, causal_out_b,
           meta_in_w, meta_in_b, meta_out_w, meta_out_b,
           out_w, out_b):
    f = np.float32
    hs = np.asarray(hidden_states, f)
    aw = np.asarray(consciousness_vector, f) @ np.asarray(aw_w, f).T \
        + np.asarray(aw_b, f)
    wfused = np.asarray(meta_out_w, f).T @ np.asarray(out_w, f).T  # [D, D]
    xTs = [np.ascontiguousarray(hs[b].T) for b in range(B)]

    def bfT(a):  # transpose + bf16
        return np.ascontiguousarray(np.asarray(a, f).T).astype(BF)

    def f8T(a, scale=16.0):  # transpose + scale + fp8
        return np.ascontiguousarray(np.asarray(a, f).T * scale).astype(F8NP)

    def pack8(a):  # [1024, cols] -> [128, 8, cols] partition-major
        return np.ascontiguousarray(
            np.asarray(a).reshape(8, 128, -1).transpose(1, 0, 2))

    def pack2(a):  # [256, 1024] -> [128, 2, 1024] partition-major
        return np.ascontiguousarray(
            np.asarray(a).reshape(2, 128, -1).transpose(1, 0, 2))

    in_maps = []
    for c in range(NCORES):
        b, g = c // G, c % G
        sl = slice(g * BAND, (g + 1) * BAND)
        wv_aug = np.zeros((D, 320), f)
        for h in range(4):
            wv_aug[:, h * 80:h * 80 + 64] = \
                16.0 * np.asarray(wv, f)[g * BAND + h * 64:
                                         g * BAND + (h + 1) * 64].T
        gw_aug = np.zeros((D, 16), f)
        gw_aug[:, 0:4] = 16.0 * np.asarray(gate_w, f)[4 * g:4 * g + 4].T
        sel4 = np.zeros((4, 512), f)
        for h in range(4):
            sel4[h, h * 128:(h + 1) * 128] = 1.0
        sel4 = sel4.astype(BF)
        pc = np.zeros((BAND, D), f)
        pc[np.arange(BAND), g * BAND + np.arange(BAND)] = 0.0625
        in_maps.append({
            "xT": xTs[b].astype(BF),
            "xf8T": pack8(xTs[b].astype(F8NP)),
            "wqT": pack8(f8T(np.asarray(wq, f)[sl])),
            "wkT": pack8(f8T(np.asarray(wk, f)[sl])),
            "wvT": pack8(wv_aug.astype(F8NP)),
            "gwT": pack8(gw_aug.astype(F8NP)),
            "selT": sel4,
            "awc": np.ascontiguousarray(aw[4 * g:4 * g + 4].reshape(1, 4)),
            "cqT": pack8(bfT(np.asarray(causal_in_w, f)[0:D][sl] / 16.0)),
            "ckT": pack8(bfT(np.asarray(causal_in_w, f)[D:2 * D][sl])),
            "cvT": pack8(bfT(np.asarray(causal_in_w, f)[2 * D:][sl])),
            "cowT": pack2(np.ascontiguousarray(
                CAUSAL_ACTIVE * np.asarray(causal_out_w, f).T[sl]).astype(BF)),
            "pcT": pack2(pc.astype(F8NP)),
            "mqT": pack8(f8T(np.asarray(meta_in_w, f)[0:D][sl])),
            "mkT": pack8(f8T(np.asarray(meta_in_w, f)[D:2 * D][sl])),
            "mvT": pack8(f8T(np.asarray(meta_in_w, f)[2 * D:][sl])),
            "mowT": pack2(np.ascontiguousarray(MW * wfused[sl]).astype(BF)),
            "owT": pack2(np.ascontiguousarray(
                (1.0 - MW) * np.asarray(out_w, f).T[sl]).astype(BF)),
            "idT": np.eye(128, dtype=np.float32).astype(BF),
        })

    nc = _get_nc()
    res = run_bass_kernel_spmd(nc, in_maps, core_ids=list(range(NCORES)))

    bias_row = (np.asarray(out_b, f)
                + MW * (np.asarray(meta_out_b, f) @ np.asarray(out_w, f).T))
    out = np.empty((B, S, D), f)
    for b in range(B):
        acc = np.zeros((D, S), f)
        for g in range(G):
            acc += res.results[b * G + g]["outP"].astype(f)
        out[b] = acc.T + bias_row[None, :]
    return out

